# revision 15
# baseline (speedup 1.0000x reference)
"""FrequencyStream Trainium2 kernel (8 NeuronCores, SPMD), v2.

Pipeline per reference nn.Module:
  x [32,3,224,224] -> 2D DCT-II per channel -> conv3x3(3->64)+relu+maxpool2
  -> conv3x3(64->128)+relu+maxpool2 -> conv3x3(128->256)+relu+maxpool2
  -> flatten -> fc [512, 200704] -> [32, 512]

Distribution: DCT+convs data-parallel over batch (4 images/core); FC
tensor-parallel over the contraction via AllToAll (host sums partials).

v2 design vs v1:
  - bf16 weights/activations after DCT pass 1 (DVE 2-4x perf modes, half DMA)
  - conv1 as block-diagonal K=56 matmul (2 row-pairs at once, 128 psum rows,
    bias via ones-rows in the im2col)
  - maxpool via tensor_tensor max pairs (+ scalar_tensor_tensor relu fusion)
    instead of TensorReduce (which has no DVE perf modes)
  - act1 row-dual built by 2 image-level strided copies, not per-tile
  - FC: one XBAR dma-transpose of h, weights stationary, out [128o, 32i]
"""

import numpy as np

_CACHE = {}

N = 224
P = 112
NIMG = 4
NCORES = 8
KCH = 25088            # fc contraction chunk per core
KT = KCH // 128        # 196 fc k-tiles per core
FPW = 260              # padded freq row width (256-wide im2col loads)
FPH = 228              # padded freq rows (226 + 2 slack for shifted dup loads)
RBLK = 28              # conv1 rows per im2col block
NBLK = N // RBLK       # 4
WRES = 48              # fc weight k-tiles resident in SBUF (prefetched)


def _dct_matrix(n):
    k = np.arange(n)[:, None].astype(np.float64)
    m = np.arange(n)[None, :].astype(np.float64)
    D = np.sqrt(2.0 / n) * np.cos(np.pi * (2 * m + 1) * k / (2 * n))
    D[0, :] *= 1.0 / np.sqrt(2.0)
    return D.astype(np.float32)


def _build(sim_single=False):
    import concourse.bass as bass
    import concourse.tile as tile
    from concourse import bacc, mybir

    F32 = mybir.dt.float32
    BF16 = mybir.dt.bfloat16
    R = mybir.dt.float32r
    MAX = mybir.AluOpType.max
    RELU = mybir.ActivationFunctionType.Relu

    nc = bacc.Bacc("TRN2", target_bir_lowering=False, debug=False,
                   num_devices=1 if sim_single else NCORES)

    x4 = nc.dram_tensor("x4", (NIMG, 3, N, N), R, kind="ExternalInput").ap()
    dctT = nc.dram_tensor("dctT", (P, 2, 256), R, kind="ExternalInput").ap()
    dct16 = nc.dram_tensor("dct16", (P, 2, 256), BF16,
                           kind="ExternalInput").ap()
    w1d = nc.dram_tensor("w1d", (56, 128), BF16, kind="ExternalInput").ap()
    w2pd = nc.dram_tensor("w2pd", (128, 3, 128), BF16,
                          kind="ExternalInput").ap()
    w2ld = nc.dram_tensor("w2ld", (64, 3, 128), BF16,
                          kind="ExternalInput").ap()
    w3d = nc.dram_tensor("w3d", (128, 9, 256), BF16, kind="ExternalInput").ap()
    b2d = nc.dram_tensor("b2d", (128, 1), F32, kind="ExternalInput").ap()
    b3d = nc.dram_tensor("b3d", (128, 2), F32, kind="ExternalInput").ap()
    onesd = nc.dram_tensor("onesd", (2, 32, 256), BF16,
                           kind="ExternalInput").ap()
    fcwT = nc.dram_tensor("fcwT", (KCH, 512), BF16, kind="ExternalInput").ap()
    partial = nc.dram_tensor("partial", (128, 4, 32), F32,
                             kind="ExternalOutput").ap()

    cc_in = nc.dram_tensor("cc_in", (NCORES, NIMG, KCH), BF16,
                           kind="Internal").ap()
    cc_out = nc.dram_tensor("cc_out", (NCORES, NIMG, KCH), BF16,
                            kind="Internal").ap()

    with tile.TileContext(nc) as tc:
        with tc.tile_pool(name="const", bufs=1) as const, \
             tc.tile_pool(name="sbA", bufs=2) as sbA, \
             tc.tile_pool(name="r1p", bufs=3) as r1p, \
             tc.tile_pool(name="sbtmp", bufs=4) as sbtmp, \
             tc.tile_pool(name="act3p", bufs=2) as act3p, \
             tc.tile_pool(name="dramp", bufs=2, space="DRAM") as dramp, \
             tc.tile_pool(name="psD", bufs=2, space="PSUM") as psD, \
             tc.tile_pool(name="psC1", bufs=3, space="PSUM") as psC1, \
             tc.tile_pool(name="psC23", bufs=2, space="PSUM") as psC23, \
             tc.tile_pool(name="fcc", bufs=1) as fcc, \
             tc.tile_pool(name="sbfc", bufs=3) as sbfc, \
             tc.tile_pool(name="psOp", bufs=1, space="PSUM") as psOp:

            # ---- constants / persistent tiles ----
            DTt = const.tile([P, 2, 256], R)
            nc.sync.dma_start(DTt[:], dctT)
            DT16 = const.tile([P, 2, 256], BF16)
            nc.sync.dma_start(DT16[:], dct16)
            w1t = const.tile([56, 128], BF16)
            w2p = const.tile([128, 3, 128], BF16)
            w2l = const.tile([64, 3, 128], BF16)
            w3t = const.tile([128, 9, 256], BF16)
            b2t = const.tile([128, 1], F32)
            b3t = const.tile([128, 2], F32)
            zt16 = const.tile([128, FPW], BF16)
            nc.vector.memset(zt16[:], 0.0)

            # fc resident weight tiles (prefetch at kernel start)
            wres = const.tile([128, WRES, 512], BF16)
            ngrp = WRES // 4
            src = bass.AP(tensor=fcwT.tensor, offset=fcwT.offset,
                          ap=[[512, 128], [128 * 512, 4], [1, 512]])
            nc.sync.dma_start(wres[:, 0:4, :], src)
            rest = list(range(1, ngrp))
            wres_sched = {i + 1: rest[i::NIMG - 1] for i in range(NIMG - 1)}

            # persistent activations (borders zeroed once)
            act1 = const.tile([128, 114, 114], BF16)
            act2 = const.tile([128, 58, 58], BF16)
            nc.vector.memset(act1[:, 0, :], 0.0)
            nc.vector.memset(act1[:, 113, :], 0.0)
            nc.gpsimd.memset(act1[:, :, 0], 0.0)
            nc.gpsimd.memset(act1[:, :, 113], 0.0)
            nc.vector.memset(act2[:, 0, :], 0.0)
            nc.vector.memset(act2[:, 57, :], 0.0)
            nc.gpsimd.memset(act2[:, :, 0], 0.0)
            nc.gpsimd.memset(act2[:, :, 57], 0.0)

            def dct(i):
                """image i -> fp DRAM [3, FPH, FPW] bf16 (zero borders)."""
                Xt = sbA.tile([P, 3, 2, N], R, tag="X")
                nc.sync.dma_start(
                    Xt[:], x4[i].rearrange("c (kt p) n -> p c kt n", p=P))
                Tt = sbA.tile([P, 2, 3, 256], BF16, tag="T")
                cp = 0
                for c in range(3):
                    for nt in range(2):
                        ps = psD.tile([P, 256], F32, tag="dct")
                        for kt in range(2):
                            nc.tensor.matmul(
                                ps[:],
                                lhsT=Xt[:, c, kt, nt * P:(nt + 1) * P],
                                rhs=DTt[:, kt, :],
                                start=(kt == 0), stop=(kt == 1))
                        if cp % 2 == 0:
                            nc.vector.tensor_copy(Tt[:, nt, c, :], ps[:])
                        else:
                            nc.scalar.copy(Tt[:, nt, c, :], ps[:])
                        cp += 1
                fq = sbA.tile([P, 2, 3, N], BF16, tag="fq")
                for c in range(3):
                    for ht in range(2):
                        ps = psD.tile([P, 256], F32, tag="dct")
                        for kt in range(2):
                            nc.tensor.matmul(
                                ps[:, 0:N],
                                lhsT=Tt[:, kt, c, ht * P:(ht + 1) * P],
                                rhs=DT16[:, kt, 0:N],
                                start=(kt == 0), stop=(kt == 1))
                        if cp % 2 == 0:
                            nc.vector.tensor_copy(fq[:, ht, c, :], ps[:, 0:N])
                        else:
                            nc.scalar.copy(fq[:, ht, c, :], ps[:, 0:N])
                        cp += 1
                fp = dramp.tile([3, FPH, FPW], BF16, tag="freqpad")
                # borders: rows 0 and 225 full width; cols 0 and 225
                nc.scalar.dma_start(fp[:, 0, :], zt16[0:3, :])
                nc.scalar.dma_start(fp[:, 225, :], zt16[0:3, :])
                nc.scalar.dma_start(fp[:, 1:225, 0], zt16[0:3, 0:224])
                nc.scalar.dma_start(fp[:, 1:225, 225], zt16[0:3, 0:224])
                # interior per channel
                for c in range(3):
                    nc.scalar.dma_start(
                        fp[c, 1:225, 1:225].rearrange(
                            "(ht p) x -> p ht x", p=P),
                        fq[:, :, c, :])
                return fp

            TPB = RBLK // 4               # conv1 tiles per block (7)

            def conv1_block(fp, blk):
                """emit r1 im2col DMAs for one 28-row block; return tile."""
                fpap = fp[:]
                Y0 = RBLK * blk
                r1 = r1p.tile([56, 32, 256], BF16, tag="r1")
                nc.gpsimd.dma_start(r1[54:56], onesd)
                for c in range(3):
                    src = bass.AP(
                        tensor=fpap.tensor,
                        offset=fpap.offset + c * FPH * FPW + Y0 * FPW,
                        ap=[[1, 3], [FPW, 32], [1, 256]])
                    qe = [nc.sync, nc.scalar, nc.sync][c]
                    qe.dma_start(r1[3 * c:3 * c + 3], src)
                # all shifted duplicates source the ty0 load (1 dep hop):
                # partitions (b,ty) shift = 2b+ty rows
                nc.sync.dma_start(r1[9:18, 0:31], r1[0:9, 1:32])
                nc.scalar.dma_start(r1[18:27, 0:30], r1[0:9, 2:32])
                nc.sync.dma_start(r1[27:36, 0:30], r1[0:9, 2:32])
                nc.scalar.dma_start(r1[36:45, 0:29], r1[0:9, 3:32])
                nc.gpsimd.dma_start(r1[45:54, 0:28], r1[0:9, 4:32])
                return r1

            def conv1_tile(r1, t):
                g = t % TPB
                Pr = 2 * t                    # first pooled row of pair
                ps = psC1.tile([128, 448], F32, tag="cv1")
                nc.tensor.matmul(
                    ps[:], lhsT=w1t[:],
                    rhs=r1[:, 4 * g:4 * g + 2, 0:N],
                    start=True, stop=True)
                psv = ps[:].rearrange("p (r x) -> p r x", r=2)
                pat = t % 3
                if pat == 0:
                    th = sbtmp.tile([128, N], BF16, tag="th1")
                    nc.vector.tensor_tensor(
                        th[:], psv[:, 0, :], psv[:, 1, :], MAX)
                    nc.vector.scalar_tensor_tensor(
                        act1[:, 1 + Pr, 1:113],
                        th[:, 0:N:2], 0.0, th[:, 1:N:2], MAX, MAX)
                else:
                    th = sbtmp.tile([128, 2, N], BF16, tag="th2")
                    nc.scalar.activation(th[:], psv, RELU)
                    veng = nc.vector if pat == 1 else nc.gpsimd
                    tv = sbtmp.tile([128, N], BF16, tag="tv1")
                    veng.tensor_tensor(
                        tv[:], th[:, 0, :], th[:, 1, :], MAX)
                    nc.vector.tensor_tensor(
                        act1[:, 1 + Pr, 1:113],
                        tv[:, 0:N:2], tv[:, 1:N:2], MAX)

            def conv1_finish():
                nc.vector.tensor_copy(act1[0:64, 2:114:2, 1:113],
                                      act1[64:128, 1:113:2, 1:113])
                nc.vector.tensor_copy(act1[64:128, 0:114:2, 1:113],
                                      act1[0:64, 1:114:2, 1:113])

            def conv1(i, fp):
                r1s = {0: conv1_block(fp, 0), 1: conv1_block(fp, 1)}
                for t in range(56):
                    blk = t // TPB
                    if blk + 1 <= 7 and (blk + 1) not in r1s and t % TPB >= 4:
                        r1s[blk + 1] = conv1_block(fp, blk + 1)
                    conv1_tile(r1s[blk], t)
                    r1s.pop(blk - 1, None)
                conv1_finish()

            def conv2(i):
                for g in range(28):
                    y0 = 4 * g
                    ps = psC23.tile([128, 448], F32, tag="cv23")
                    for dx in range(3):
                        nc.tensor.matmul(
                            ps[:], lhsT=w2p[:, dx, :],
                            rhs=act1[:, y0:y0 + 4, dx:dx + P],
                            start=(dx == 0), stop=False)
                    for dx in range(3):
                        nc.tensor.matmul(
                            ps[:], lhsT=w2l[:, dx, :],
                            rhs=act1[0:64, y0 + 2:y0 + 6, dx:dx + P],
                            start=False, stop=(dx == 2))
                    psv = ps[:].rearrange("p (r x) -> p r x", r=4)
                    tv = sbtmp.tile([128, 2, P], BF16, tag="tv2")
                    nc.vector.tensor_tensor(
                        tv[:], psv[:, 0:4:2, :], psv[:, 1:4:2, :], MAX)
                    tm = sbtmp.tile([128, 2, 56], BF16, tag="tm2")
                    nc.vector.tensor_tensor(
                        tm[:], tv[:, :, 0:P:2], tv[:, :, 1:P:2], MAX)
                    nc.scalar.activation(act2[:, 1 + 2 * g:3 + 2 * g, 1:57],
                                         tm[:], RELU, bias=b2t[:, 0:1])

            def conv3(i, c1_fp=None):
                """conv3(i); if c1_fp, interleave conv1(i+1) tile emission."""
                act3 = act3p.tile([128, 2, 28, 28], BF16, tag="act3")
                taps9 = [(dy, dx) for dy in range(3) for dx in range(3)]
                r1s = {}
                c1t = 0
                if c1_fp is not None:
                    r1s[0] = conv1_block(c1_fp, 0)
                    r1s[1] = conv1_block(c1_fp, 1)
                for mt in range(2):
                    for g in range(7):
                        y0 = 8 * g
                        ps = psC23.tile([128, 448], F32, tag="cv23")
                        for t, (dy, dx) in enumerate(taps9):
                            nc.tensor.matmul(
                                ps[:],
                                lhsT=w3t[:, t, mt * 128:(mt + 1) * 128],
                                rhs=act2[:, y0 + dy:y0 + dy + 8, dx:dx + 56],
                                start=(t == 0), stop=(t == 8))
                        psv = ps[:].rearrange("p (r x) -> p r x", r=8)
                        tv = sbtmp.tile([128, 4, 56], BF16, tag="tv3")
                        nc.vector.tensor_tensor(
                            tv[:], psv[:, 0:8:2, :], psv[:, 1:8:2, :], MAX)
                        tm = sbtmp.tile([128, 4, 28], BF16, tag="tm3")
                        nc.vector.tensor_tensor(
                            tm[:], tv[:, :, 0:56:2], tv[:, :, 1:56:2], MAX)
                        nc.scalar.activation(act3[:, mt, 4 * g:4 * g + 4, :],
                                             tm[:], RELU,
                                             bias=b3t[:, mt:mt + 1])
                        if c1_fp is not None:
                            for _ in range(4):
                                if c1t >= 56:
                                    continue
                                blk = c1t // TPB
                                for nb in (blk + 1, blk + 2):
                                    if nb <= 7 and nb not in r1s:
                                        r1s[nb] = conv1_block(c1_fp, nb)
                                conv1_tile(r1s[blk], c1t)
                                c1t += 1
                if c1_fp is not None:
                    conv1_finish()
                return act3

            def hout(i, act3):
                # k = c_global*784 + s ; dest core d = c_global // 32
                a3 = act3[:].rearrange("p mt a b -> p mt (a b)")
                for mt in range(2):
                    dst = bass.AP(
                        tensor=cc_in.tensor,
                        offset=cc_in.offset + (4 * mt) * NIMG * KCH + i * KCH,
                        ap=[[NIMG * KCH, 4], [784, 32], [1, 784]])
                    nc.gpsimd.dma_start(dst, a3[:, mt, :])

            # ---------------- pipeline ----------------
            fp0 = dct(0)
            nc.sync.dma_start(w1t[:], w1d)
            fp1 = dct(1)
            nc.sync.dma_start(w2p[:], w2pd)
            nc.sync.dma_start(w2l[:], w2ld)
            nc.scalar.dma_start(b2t[:], b2d)
            conv1(0, fp0)
            nc.sync.dma_start(w3t[:], w3d)
            nc.scalar.dma_start(b3t[:], b3d)
            prev_fp = fp1
            for i in range(NIMG):
                conv2(i)
                for g in wres_sched.get(i, []):
                    src = bass.AP(tensor=fcwT.tensor,
                                  offset=fcwT.offset + g * 4 * 128 * 512,
                                  ap=[[512, 128], [128 * 512, 4], [1, 512]])
                    nc.scalar.dma_start(wres[:, 4 * g:4 * g + 4, :], src)
                nfp = dct(i + 2) if i + 2 < NIMG else None
                a3 = conv3(i, prev_fp if i + 1 < NIMG else None)
                hout(i, a3)
                prev_fp = nfp

            # ---------------- fc phase ----------------
            if sim_single:
                nc.sync.dma_start(cc_out, cc_in)
            else:
                nc.gpsimd.collective_compute(
                    "AllToAll", mybir.AluOpType.bypass,
                    replica_groups=[list(range(NCORES))],
                    ins=[cc_in], outs=[cc_out])

            ccv = bass.AP(tensor=cc_out.tensor, offset=cc_out.offset,
                          ap=[[KCH, 32], [1, KCH]])
            hT = fcc.tile([128, KT, 32], BF16)
            nc.sync.dma_start(hT[:], ccv, transpose=True)

            psO = psOp.tile([128, 4, 32], F32)
            wt = None
            for e in range(KT):
                if e < WRES:
                    wbase, wi = wres, e
                else:
                    if (e - WRES) % 4 == 0:
                        wt = sbfc.tile([128, 4, 512], BF16, tag="wt")
                        src = bass.AP(
                            tensor=fcwT.tensor,
                            offset=fcwT.offset + e * 128 * 512,
                            ap=[[512, 128], [128 * 512, 4], [1, 512]])
                        weng = nc.sync if (e // 4) % 2 == 0 else nc.scalar
                        weng.dma_start(wt[:], src)
                    wbase, wi = wt, (e - WRES) % 4
                for og in range(4):
                    nc.tensor.matmul(
                        psO[:, og, :],
                        lhsT=wbase[:, wi, 128 * og:128 * og + 128],
                        rhs=hT[:, e, :],
                        start=(e == 0), stop=(e == KT - 1))
            outsb = sbfc.tile([128, 4, 32], F32, tag="out")
            nc.vector.tensor_copy(outsb[:], psO[:])
            nc.sync.dma_start(partial, outsb[:])

    nc.compile()
    return nc


def _prep_inputs(x, w1, b1, w2, b2, w3, b3, fcw):
    import ml_dtypes
    BF = ml_dtypes.bfloat16
    D = _dct_matrix(N)
    DTt = np.zeros((P, 2, 256), np.float32)
    DTt[:, :, 0:N] = D.T.reshape(2, P, N).transpose(1, 0, 2)

    # conv1 block-diag weights [56, 128]: k = b*27 + ty*9 + c*3 + tx
    w1 = np.asarray(w1, np.float32)          # [o, c, ty, tx]
    w1k = w1.transpose(1, 2, 3, 0).reshape(27, 64)   # [(c,ty,tx), o]
    idx = np.array([c * 9 + ty * 3 + tx
                    for ty in range(3) for c in range(3) for tx in range(3)])
    w1k = w1k[idx]                                   # [(ty,c,tx), o]
    b1 = np.asarray(b1, np.float32)
    w1bd = np.zeros((56, 128), np.float32)
    w1bd[0:27, 0:64] = w1k
    w1bd[27:54, 64:128] = w1k
    w1bd[54, 0:64] = b1
    w1bd[55, 64:128] = b1

    w2 = np.asarray(w2, np.float32)
    w2pair = np.empty((128, 3, 128), np.float32)
    w2last = np.empty((64, 3, 128), np.float32)
    for dx in range(3):
        w2pair[0:64, dx, :] = w2[:, :, 0, dx].T
        w2pair[64:128, dx, :] = w2[:, :, 1, dx].T
        w2last[:, dx, :] = w2[:, :, 2, dx].T
    w3sb = np.ascontiguousarray(
        np.asarray(w3, np.float32).transpose(1, 2, 3, 0).reshape(128, 9, 256))
    b3sb = np.ascontiguousarray(np.asarray(b3, np.float32).reshape(2, 128).T)

    x = np.ascontiguousarray(np.asarray(x, np.float32))
    fcw8 = np.asarray(fcw, np.float32).reshape(512, NCORES, KCH)

    ones = np.ones((2, 32, 256), np.float32).astype(BF)
    in_maps = []
    for j in range(NCORES):
        fcwT_j = np.ascontiguousarray(fcw8[:, j, :].T).astype(BF)
        in_maps.append({
            "x4": x[4 * j:4 * j + 4],
            "dctT": DTt,
            "dct16": DTt.astype(BF),
            "w1d": w1bd.astype(BF),
            "w2pd": w2pair.astype(BF),
            "w2ld": w2last.astype(BF),
            "w3d": w3sb.astype(BF),
            "b2d": np.ascontiguousarray(np.asarray(b2, np.float32)[:, None]),
            "b3d": b3sb,
            "onesd": ones,
            "fcwT": fcwT_j,
        })
    return in_maps


def kernel(x, w1, b1, w2, b2, w3, b3, fcw, fcb, _trace=False):
    from concourse import bass_utils

    if "nc" not in _CACHE:
        _CACHE["nc"] = _build()
    nc = _CACHE["nc"]

    in_maps = _prep_inputs(x, w1, b1, w2, b2, w3, b3, fcw)
    res = bass_utils.run_bass_kernel_spmd(
        nc, in_maps, core_ids=list(range(NCORES)), trace=_trace)
    out = np.zeros((32, 512), np.float32)
    for j in range(NCORES):
        pj = np.asarray(res.results[j]["partial"], np.float32)
        out += pj.transpose(2, 1, 0).reshape(32, 512)
    out += np.asarray(fcb, np.float32)[None, :]
    if _trace:
        return out, res
    return out


# revision 16
# speedup vs baseline: 1.1068x; 1.1068x over previous
"""FrequencyStream Trainium2 kernel (8 NeuronCores, SPMD), v2.

Pipeline per reference nn.Module:
  x [32,3,224,224] -> 2D DCT-II per channel -> conv3x3(3->64)+relu+maxpool2
  -> conv3x3(64->128)+relu+maxpool2 -> conv3x3(128->256)+relu+maxpool2
  -> flatten -> fc [512, 200704] -> [32, 512]

Distribution: DCT+convs data-parallel over batch (4 images/core); FC
tensor-parallel over the contraction via AllToAll (host sums partials).

v2 design vs v1:
  - bf16 weights/activations after DCT pass 1 (DVE 2-4x perf modes, half DMA)
  - conv1 as block-diagonal K=56 matmul (2 row-pairs at once, 128 psum rows,
    bias via ones-rows in the im2col)
  - maxpool via tensor_tensor max pairs (+ scalar_tensor_tensor relu fusion)
    instead of TensorReduce (which has no DVE perf modes)
  - act1 row-dual built by 2 image-level strided copies, not per-tile
  - FC: one XBAR dma-transpose of h, weights stationary, out [128o, 32i]
"""

import numpy as np

_CACHE = {}

N = 224
P = 112
NIMG = 4
NCORES = 8
KCH = 25088            # fc contraction chunk per core
KT = KCH // 128        # 196 fc k-tiles per core
FPW = 260              # padded freq row width (256-wide im2col loads)
FPH = 228              # padded freq rows (226 + 2 slack for shifted dup loads)
RBLK = 56              # conv1 rows per im2col block
NBLK = N // RBLK       # 4
WRES = 28              # fc weight k-tiles resident in SBUF (prefetched)


def _dct_matrix(n):
    k = np.arange(n)[:, None].astype(np.float64)
    m = np.arange(n)[None, :].astype(np.float64)
    D = np.sqrt(2.0 / n) * np.cos(np.pi * (2 * m + 1) * k / (2 * n))
    D[0, :] *= 1.0 / np.sqrt(2.0)
    return D.astype(np.float32)


def _build(sim_single=False):
    import concourse.bass as bass
    import concourse.tile as tile
    from concourse import bacc, mybir

    F32 = mybir.dt.float32
    BF16 = mybir.dt.bfloat16
    R = mybir.dt.float32r
    MAX = mybir.AluOpType.max
    RELU = mybir.ActivationFunctionType.Relu

    nc = bacc.Bacc("TRN2", target_bir_lowering=False, debug=False,
                   num_devices=1 if sim_single else NCORES)

    x4 = nc.dram_tensor("x4", (NIMG, 3, N, N), R, kind="ExternalInput").ap()
    dctT = nc.dram_tensor("dctT", (P, 2, 256), R, kind="ExternalInput").ap()
    dct16 = nc.dram_tensor("dct16", (P, 2, 256), BF16,
                           kind="ExternalInput").ap()
    w1d = nc.dram_tensor("w1d", (56, 128), BF16, kind="ExternalInput").ap()
    w2pd = nc.dram_tensor("w2pd", (128, 3, 128), BF16,
                          kind="ExternalInput").ap()
    w2ld = nc.dram_tensor("w2ld", (64, 3, 128), BF16,
                          kind="ExternalInput").ap()
    w3d = nc.dram_tensor("w3d", (128, 9, 256), BF16, kind="ExternalInput").ap()
    b2d = nc.dram_tensor("b2d", (128, 1), F32, kind="ExternalInput").ap()
    b3d = nc.dram_tensor("b3d", (128, 2), F32, kind="ExternalInput").ap()
    onesd = nc.dram_tensor("onesd", (2, 60, 256), BF16,
                           kind="ExternalInput").ap()
    fcwT = nc.dram_tensor("fcwT", (KCH, 512), BF16, kind="ExternalInput").ap()
    partial = nc.dram_tensor("partial", (128, 4, 32), F32,
                             kind="ExternalOutput").ap()

    cc_in = nc.dram_tensor("cc_in", (NCORES, NIMG, KCH), BF16,
                           kind="Internal").ap()
    cc_out = nc.dram_tensor("cc_out", (NCORES, NIMG, KCH), BF16,
                            kind="Internal").ap()

    with tile.TileContext(nc) as tc:
        with tc.tile_pool(name="const", bufs=1) as const, \
             tc.tile_pool(name="sbA", bufs=2) as sbA, \
             tc.tile_pool(name="r1p", bufs=2) as r1p, \
             tc.tile_pool(name="sbtmp", bufs=4) as sbtmp, \
             tc.tile_pool(name="act3p", bufs=2) as act3p, \
             tc.tile_pool(name="dramp", bufs=2, space="DRAM") as dramp, \
             tc.tile_pool(name="psD", bufs=2, space="PSUM") as psD, \
             tc.tile_pool(name="psC1", bufs=3, space="PSUM") as psC1, \
             tc.tile_pool(name="psC23", bufs=2, space="PSUM") as psC23, \
             tc.tile_pool(name="fcc", bufs=1) as fcc, \
             tc.tile_pool(name="sbfc", bufs=3) as sbfc, \
             tc.tile_pool(name="psOp", bufs=1, space="PSUM") as psOp:

            # ---- constants / persistent tiles ----
            DTt = const.tile([P, 2, 256], R)
            nc.sync.dma_start(DTt[:], dctT)
            DT16 = const.tile([P, 2, 256], BF16)
            nc.sync.dma_start(DT16[:], dct16)
            w1t = const.tile([56, 128], BF16)
            w2p = const.tile([128, 3, 128], BF16)
            w2l = const.tile([64, 3, 128], BF16)
            w3t = const.tile([128, 9, 256], BF16)
            b2t = const.tile([128, 1], F32)
            b3t = const.tile([128, 2], F32)
            zt16 = const.tile([128, FPW], BF16)
            nc.vector.memset(zt16[:], 0.0)

            # fc resident weight tiles (prefetch at kernel start)
            wres = const.tile([128, WRES, 512], BF16)
            ngrp = WRES // 4
            src = bass.AP(tensor=fcwT.tensor, offset=fcwT.offset,
                          ap=[[512, 128], [128 * 512, 4], [1, 512]])
            nc.sync.dma_start(wres[:, 0:4, :], src)
            rest = list(range(1, ngrp))
            wres_sched = {i + 1: rest[i::NIMG - 1] for i in range(NIMG - 1)}

            # persistent activations (borders zeroed once)
            act1 = const.tile([128, 114, 114], BF16)
            act2 = const.tile([128, 58, 58], BF16)
            nc.vector.memset(act1[:, 0, :], 0.0)
            nc.vector.memset(act1[:, 113, :], 0.0)
            nc.gpsimd.memset(act1[:, :, 0], 0.0)
            nc.gpsimd.memset(act1[:, :, 113], 0.0)
            nc.vector.memset(act2[:, 0, :], 0.0)
            nc.vector.memset(act2[:, 57, :], 0.0)
            nc.gpsimd.memset(act2[:, :, 0], 0.0)
            nc.gpsimd.memset(act2[:, :, 57], 0.0)

            def dct(i):
                """image i -> fp DRAM [3, FPH, FPW] bf16 (zero borders)."""
                Xt = sbA.tile([P, 3, 2, N], R, tag="X")
                nc.sync.dma_start(
                    Xt[:], x4[i].rearrange("c (kt p) n -> p c kt n", p=P))
                Tt = sbA.tile([P, 2, 3, 256], BF16, tag="T")
                cp = 0
                for c in range(3):
                    for nt in range(2):
                        ps = psD.tile([P, 256], F32, tag="dct")
                        for kt in range(2):
                            nc.tensor.matmul(
                                ps[:],
                                lhsT=Xt[:, c, kt, nt * P:(nt + 1) * P],
                                rhs=DTt[:, kt, :],
                                start=(kt == 0), stop=(kt == 1))
                        if cp % 2 == 0:
                            nc.vector.tensor_copy(Tt[:, nt, c, :], ps[:])
                        else:
                            nc.scalar.copy(Tt[:, nt, c, :], ps[:])
                        cp += 1
                fq = sbA.tile([P, 2, 3, N], BF16, tag="fq")
                for c in range(3):
                    for ht in range(2):
                        ps = psD.tile([P, 256], F32, tag="dct")
                        for kt in range(2):
                            nc.tensor.matmul(
                                ps[:, 0:N],
                                lhsT=Tt[:, kt, c, ht * P:(ht + 1) * P],
                                rhs=DT16[:, kt, 0:N],
                                start=(kt == 0), stop=(kt == 1))
                        if cp % 2 == 0:
                            nc.vector.tensor_copy(fq[:, ht, c, :], ps[:, 0:N])
                        else:
                            nc.scalar.copy(fq[:, ht, c, :], ps[:, 0:N])
                        cp += 1
                fp = dramp.tile([3, FPH, FPW], BF16, tag="freqpad")
                # borders: rows 0 and 225 full width; cols 0 and 225
                nc.scalar.dma_start(fp[:, 0, :], zt16[0:3, :])
                nc.scalar.dma_start(fp[:, 225, :], zt16[0:3, :])
                nc.scalar.dma_start(fp[:, 1:225, 0], zt16[0:3, 0:224])
                nc.scalar.dma_start(fp[:, 1:225, 225], zt16[0:3, 0:224])
                # interior per channel
                for c in range(3):
                    nc.scalar.dma_start(
                        fp[c, 1:225, 1:225].rearrange(
                            "(ht p) x -> p ht x", p=P),
                        fq[:, :, c, :])
                return fp

            TPB = RBLK // 4               # conv1 tiles per block (7)

            def conv1_block(fp, blk):
                """emit r1 im2col DMAs for one 28-row block; return tile."""
                fpap = fp[:]
                Y0 = RBLK * blk
                r1 = r1p.tile([56, 60, 256], BF16, tag="r1")
                nc.gpsimd.dma_start(r1[54:56], onesd)
                for c in range(3):
                    src = bass.AP(
                        tensor=fpap.tensor,
                        offset=fpap.offset + c * FPH * FPW + Y0 * FPW,
                        ap=[[1, 3], [FPW, 60], [1, 256]])
                    qe = [nc.sync, nc.scalar, nc.sync][c]
                    qe.dma_start(r1[3 * c:3 * c + 3], src)
                # all shifted duplicates source the ty0 load (1 dep hop):
                # partitions (b,ty) shift = 2b+ty rows
                nc.sync.dma_start(r1[9:18, 0:59], r1[0:9, 1:60])
                nc.scalar.dma_start(r1[18:27, 0:58], r1[0:9, 2:60])
                nc.sync.dma_start(r1[27:36, 0:58], r1[0:9, 2:60])
                nc.scalar.dma_start(r1[36:45, 0:57], r1[0:9, 3:60])
                nc.gpsimd.dma_start(r1[45:54, 0:56], r1[0:9, 4:60])
                return r1

            def conv1_tile(r1, t):
                g = t % TPB
                Pr = 2 * t                    # first pooled row of pair
                ps = psC1.tile([128, 448], F32, tag="cv1")
                nc.tensor.matmul(
                    ps[:], lhsT=w1t[:],
                    rhs=r1[:, 4 * g:4 * g + 2, 0:N],
                    start=True, stop=True)
                psv = ps[:].rearrange("p (r x) -> p r x", r=2)
                pat = t % 3
                if pat == 0:
                    th = sbtmp.tile([128, N], BF16, tag="th1")
                    nc.vector.tensor_tensor(
                        th[:], psv[:, 0, :], psv[:, 1, :], MAX)
                    nc.vector.scalar_tensor_tensor(
                        act1[:, 1 + Pr, 1:113],
                        th[:, 0:N:2], 0.0, th[:, 1:N:2], MAX, MAX)
                else:
                    th = sbtmp.tile([128, 2, N], BF16, tag="th2")
                    nc.scalar.activation(th[:], psv, RELU)
                    veng = nc.vector if pat == 1 else nc.gpsimd
                    tv = sbtmp.tile([128, N], BF16, tag="tv1")
                    veng.tensor_tensor(
                        tv[:], th[:, 0, :], th[:, 1, :], MAX)
                    nc.vector.tensor_tensor(
                        act1[:, 1 + Pr, 1:113],
                        tv[:, 0:N:2], tv[:, 1:N:2], MAX)

            def conv1_finish():
                nc.vector.tensor_copy(act1[0:64, 2:114:2, 1:113],
                                      act1[64:128, 1:113:2, 1:113])
                nc.vector.tensor_copy(act1[64:128, 0:114:2, 1:113],
                                      act1[0:64, 1:114:2, 1:113])

            def conv1(i, fp):
                r1s = {0: conv1_block(fp, 0)}
                for t in range(56):
                    blk = t // TPB
                    if (blk + 1 < NBLK and (blk + 1) not in r1s
                            and t % TPB >= 7):
                        r1s[blk + 1] = conv1_block(fp, blk + 1)
                    conv1_tile(r1s[blk], t)
                conv1_finish()

            def conv2(i):
                for g in range(28):
                    y0 = 4 * g
                    ps = psC23.tile([128, 448], F32, tag="cv23")
                    for dx in range(3):
                        nc.tensor.matmul(
                            ps[:], lhsT=w2p[:, dx, :],
                            rhs=act1[:, y0:y0 + 4, dx:dx + P],
                            start=(dx == 0), stop=False)
                    for dx in range(3):
                        nc.tensor.matmul(
                            ps[:], lhsT=w2l[:, dx, :],
                            rhs=act1[0:64, y0 + 2:y0 + 6, dx:dx + P],
                            start=False, stop=(dx == 2))
                    psv = ps[:].rearrange("p (r x) -> p r x", r=4)
                    tv = sbtmp.tile([128, 2, P], BF16, tag="tv2")
                    nc.vector.tensor_tensor(
                        tv[:], psv[:, 0:4:2, :], psv[:, 1:4:2, :], MAX)
                    tm = sbtmp.tile([128, 2, 56], BF16, tag="tm2")
                    nc.vector.tensor_tensor(
                        tm[:], tv[:, :, 0:P:2], tv[:, :, 1:P:2], MAX)
                    nc.scalar.activation(act2[:, 1 + 2 * g:3 + 2 * g, 1:57],
                                         tm[:], RELU, bias=b2t[:, 0:1])

            def conv3(i, c1_fp=None):
                """conv3(i); if c1_fp, interleave conv1(i+1) tile emission."""
                act3 = act3p.tile([128, 2, 28, 28], BF16, tag="act3")
                taps9 = [(dy, dx) for dy in range(3) for dx in range(3)]
                r1s = {}
                c1t = 0
                if c1_fp is not None:
                    r1s[0] = conv1_block(c1_fp, 0)
                for mt in range(2):
                    for g in range(7):
                        y0 = 8 * g
                        ps = psC23.tile([128, 448], F32, tag="cv23")
                        for t, (dy, dx) in enumerate(taps9):
                            nc.tensor.matmul(
                                ps[:],
                                lhsT=w3t[:, t, mt * 128:(mt + 1) * 128],
                                rhs=act2[:, y0 + dy:y0 + dy + 8, dx:dx + 56],
                                start=(t == 0), stop=(t == 8))
                        psv = ps[:].rearrange("p (r x) -> p r x", r=8)
                        tv = sbtmp.tile([128, 4, 56], BF16, tag="tv3")
                        nc.vector.tensor_tensor(
                            tv[:], psv[:, 0:8:2, :], psv[:, 1:8:2, :], MAX)
                        tm = sbtmp.tile([128, 4, 28], BF16, tag="tm3")
                        nc.vector.tensor_tensor(
                            tm[:], tv[:, :, 0:56:2], tv[:, :, 1:56:2], MAX)
                        nc.scalar.activation(act3[:, mt, 4 * g:4 * g + 4, :],
                                             tm[:], RELU,
                                             bias=b3t[:, mt:mt + 1])
                        if c1_fp is not None:
                            for _ in range(4):
                                if c1t >= 56:
                                    continue
                                blk = c1t // TPB
                                nb = blk + 1
                                if (nb < NBLK and nb not in r1s
                                        and c1t % TPB >= 7):
                                    r1s[nb] = conv1_block(c1_fp, nb)
                                conv1_tile(r1s[blk], c1t)
                                c1t += 1
                if c1_fp is not None:
                    conv1_finish()
                return act3

            def hout(i, act3):
                # k = c_global*784 + s ; dest core d = c_global // 32
                a3 = act3[:].rearrange("p mt a b -> p mt (a b)")
                for mt in range(2):
                    dst = bass.AP(
                        tensor=cc_in.tensor,
                        offset=cc_in.offset + (4 * mt) * NIMG * KCH + i * KCH,
                        ap=[[NIMG * KCH, 4], [784, 32], [1, 784]])
                    nc.gpsimd.dma_start(dst, a3[:, mt, :])

            # ---------------- pipeline ----------------
            fp0 = dct(0)
            nc.sync.dma_start(w1t[:], w1d)
            fp1 = dct(1)
            nc.sync.dma_start(w2p[:], w2pd)
            nc.sync.dma_start(w2l[:], w2ld)
            nc.scalar.dma_start(b2t[:], b2d)
            conv1(0, fp0)
            nc.sync.dma_start(w3t[:], w3d)
            nc.scalar.dma_start(b3t[:], b3d)
            prev_fp = fp1
            for i in range(NIMG):
                conv2(i)
                for g in wres_sched.get(i, []):
                    src = bass.AP(tensor=fcwT.tensor,
                                  offset=fcwT.offset + g * 4 * 128 * 512,
                                  ap=[[512, 128], [128 * 512, 4], [1, 512]])
                    nc.scalar.dma_start(wres[:, 4 * g:4 * g + 4, :], src)
                nfp = dct(i + 2) if i + 2 < NIMG else None
                a3 = conv3(i, prev_fp if i + 1 < NIMG else None)
                hout(i, a3)
                prev_fp = nfp

            # ---------------- fc phase ----------------
            if sim_single:
                nc.sync.dma_start(cc_out, cc_in)
            else:
                nc.gpsimd.collective_compute(
                    "AllToAll", mybir.AluOpType.bypass,
                    replica_groups=[list(range(NCORES))],
                    ins=[cc_in], outs=[cc_out])

            ccv = bass.AP(tensor=cc_out.tensor, offset=cc_out.offset,
                          ap=[[KCH, 32], [1, KCH]])
            hT = fcc.tile([128, KT, 32], BF16)
            nc.sync.dma_start(hT[:], ccv, transpose=True)

            psO = psOp.tile([128, 4, 32], F32)
            wt = None
            for e in range(KT):
                if e < WRES:
                    wbase, wi = wres, e
                else:
                    if (e - WRES) % 4 == 0:
                        wt = sbfc.tile([128, 4, 512], BF16, tag="wt")
                        src = bass.AP(
                            tensor=fcwT.tensor,
                            offset=fcwT.offset + e * 128 * 512,
                            ap=[[512, 128], [128 * 512, 4], [1, 512]])
                        weng = nc.sync if (e // 4) % 2 == 0 else nc.scalar
                        weng.dma_start(wt[:], src)
                    wbase, wi = wt, (e - WRES) % 4
                for og in range(4):
                    nc.tensor.matmul(
                        psO[:, og, :],
                        lhsT=wbase[:, wi, 128 * og:128 * og + 128],
                        rhs=hT[:, e, :],
                        start=(e == 0), stop=(e == KT - 1))
            outsb = sbfc.tile([128, 4, 32], F32, tag="out")
            nc.vector.tensor_copy(outsb[:], psO[:])
            nc.sync.dma_start(partial, outsb[:])

    nc.compile()
    return nc


def _prep_inputs(x, w1, b1, w2, b2, w3, b3, fcw):
    import ml_dtypes
    BF = ml_dtypes.bfloat16
    D = _dct_matrix(N)
    DTt = np.zeros((P, 2, 256), np.float32)
    DTt[:, :, 0:N] = D.T.reshape(2, P, N).transpose(1, 0, 2)

    # conv1 block-diag weights [56, 128]: k = b*27 + ty*9 + c*3 + tx
    w1 = np.asarray(w1, np.float32)          # [o, c, ty, tx]
    w1k = w1.transpose(1, 2, 3, 0).reshape(27, 64)   # [(c,ty,tx), o]
    idx = np.array([c * 9 + ty * 3 + tx
                    for ty in range(3) for c in range(3) for tx in range(3)])
    w1k = w1k[idx]                                   # [(ty,c,tx), o]
    b1 = np.asarray(b1, np.float32)
    w1bd = np.zeros((56, 128), np.float32)
    w1bd[0:27, 0:64] = w1k
    w1bd[27:54, 64:128] = w1k
    w1bd[54, 0:64] = b1
    w1bd[55, 64:128] = b1

    w2 = np.asarray(w2, np.float32)
    w2pair = np.empty((128, 3, 128), np.float32)
    w2last = np.empty((64, 3, 128), np.float32)
    for dx in range(3):
        w2pair[0:64, dx, :] = w2[:, :, 0, dx].T
        w2pair[64:128, dx, :] = w2[:, :, 1, dx].T
        w2last[:, dx, :] = w2[:, :, 2, dx].T
    w3sb = np.ascontiguousarray(
        np.asarray(w3, np.float32).transpose(1, 2, 3, 0).reshape(128, 9, 256))
    b3sb = np.ascontiguousarray(np.asarray(b3, np.float32).reshape(2, 128).T)

    x = np.ascontiguousarray(np.asarray(x, np.float32))
    fcw8 = np.asarray(fcw, np.float32).reshape(512, NCORES, KCH)

    ones = np.ones((2, 60, 256), np.float32).astype(BF)
    in_maps = []
    for j in range(NCORES):
        fcwT_j = np.ascontiguousarray(fcw8[:, j, :].T).astype(BF)
        in_maps.append({
            "x4": x[4 * j:4 * j + 4],
            "dctT": DTt,
            "dct16": DTt.astype(BF),
            "w1d": w1bd.astype(BF),
            "w2pd": w2pair.astype(BF),
            "w2ld": w2last.astype(BF),
            "w3d": w3sb.astype(BF),
            "b2d": np.ascontiguousarray(np.asarray(b2, np.float32)[:, None]),
            "b3d": b3sb,
            "onesd": ones,
            "fcwT": fcwT_j,
        })
    return in_maps


def kernel(x, w1, b1, w2, b2, w3, b3, fcw, fcb, _trace=False):
    from concourse import bass_utils

    if "nc" not in _CACHE:
        _CACHE["nc"] = _build()
    nc = _CACHE["nc"]

    in_maps = _prep_inputs(x, w1, b1, w2, b2, w3, b3, fcw)
    res = bass_utils.run_bass_kernel_spmd(
        nc, in_maps, core_ids=list(range(NCORES)), trace=_trace)
    out = np.zeros((32, 512), np.float32)
    for j in range(NCORES):
        pj = np.asarray(res.results[j]["partial"], np.float32)
        out += pj.transpose(2, 1, 0).reshape(32, 512)
    out += np.asarray(fcb, np.float32)[None, :]
    if _trace:
        return out, res
    return out
